# revision 2
# baseline (speedup 1.0000x reference)
"""Trainium2 Bass kernel v3 for a 4-layer IndRNN (B=32, T=2048, I=256, H=512).

v5 = v4 + startup trim: wbc packed into one [128, L*m4] tile (1 DMA instead
of 16), and input DMAs for the first batch pair issued before the later-layer
weight loads so the first scan starts ~20us earlier.
v4 = v3 + PAIR-wise layer-major tile order (for batch-pair: for l: for b
in pair: for m). The per-layer serialization chain relu(l, m=3) -> matmul
group -> scan1(l+1, m=0) is hidden behind the partner batch's ~40us of DVE
work, removing ~100us of DVE idle at layer boundaries, while h tiles span
only 2 batches x 2 layer generations (16 bufs). v3: TT-subtract combine
(DVE 2x) + ACT final relu.

Math: per layer, xp = x @ W.T + b, then the per-channel recurrence
    h_t = relu(xp_t + w * h_{t-1}),  w = whs[l] in [0, 1)

Since w >= 0, the nonlinear scan decomposes into two linear-style scans that
map 1:1 onto the DVE `tensor_tensor_scan` instruction:
    dloc_t = w * dloc_{t-1} + xp_t          (unclamped linear scan)
    q_t    = w * min(dloc_{t-1}, q_{t-1})   (min-scan; dloc_{-1} = q_{-1} = 0)
    h_t    = relu(dloc_t - q_t)
Proof sketch: with s_t = xp_t + w*relu(s_{t-1}) (so h_t = relu(s_t)),
s_t = max(xp_t, xp_t + w*s_{t-1}) for w >= 0; substituting s_t = dloc_t + r_t
gives r_t = max(-w*dloc_{t-1}, w*r_{t-1}), i.e. q_t = -r_t satisfies the
min-scan above. Verified exactly in fp64 against the sequential reference.

Sharding: data-parallel over batch, 4 batches per core, weights replicated.
Layout on device: [H(partitions), T(free)] per batch; the host pre-transposes
the layer-0 input to [I, T] and post-transposes the output from [H, T], so the
device never pays for transposes.
"""

import numpy as np
from contextlib import ExitStack

import concourse.bass as bass
import concourse.tile as tile
from concourse import mybir
from concourse.bass_utils import run_bass_kernel_spmd

dt = mybir.dt
Alu = mybir.AluOpType
Act = mybir.ActivationFunctionType

B, T, I, H, L = 32, 2048, 256, 512, 4
NCORES = 8
BLOC = B // NCORES
P = 128
TCH = 512  # time chunk = one PSUM bank of fp32


def build(bloc=BLOC, t=T, include_bias=False, trace_sim=False,
          whole_t_scan=True, bcast_w=True, gpsimd_stt=False, act_relu=True):
    """Build the per-core Bass program (SPMD; identical on all cores)."""
    assert t % TCH == 0
    nch = t // TCH
    ki, kh, m4 = I // P, H // P, H // P

    nc = bass.Bass("TRN2", target_bir_lowering=False, debug=False,
                   num_devices=NCORES)
    xT_d = nc.dram_tensor("xT", [bloc, I, t], dt.float16, kind="ExternalInput").ap()
    w0t_d = nc.dram_tensor("w0t", [I, H], dt.float16, kind="ExternalInput").ap()
    wst_d = nc.dram_tensor("wst", [L - 1, H, H], dt.float16, kind="ExternalInput").ap()
    bias_d = nc.dram_tensor("bias", [L, 1, H], dt.float16, kind="ExternalInput").ap()
    wbc_d = nc.dram_tensor("wbc", [P, L * (H // P)], dt.float32, kind="ExternalInput").ap()
    out_d = nc.dram_tensor("out", [bloc, H, t], dt.float16, kind="ExternalOutput").ap()

    with tile.TileContext(nc, trace_sim=trace_sim) as tc, ExitStack() as ctx:
        wpool = ctx.enter_context(tc.tile_pool(name="weights", bufs=1))
        xpool = ctx.enter_context(tc.tile_pool(name="xin", bufs=2 * BLOC))
        hpool = ctx.enter_context(tc.tile_pool(name="h", bufs=16))
        dpool = ctx.enter_context(tc.tile_pool(name="dloc", bufs=3))
        qpool = ctx.enter_context(tc.tile_pool(name="q", bufs=3))
        spool = ctx.enter_context(tc.tile_pool(name="s", bufs=3))
        opool = ctx.enter_context(tc.tile_pool(name="hout", bufs=4))
        psum = ctx.enter_context(tc.tile_pool(name="psum", bufs=2, space="PSUM"))

        # --- persistent weights ---
        # layer-0 weights first so the first matmul group can start ASAP;
        # later-layer weights stream in behind it.
        wt = []  # wt[l][k] -> [128, H] fp16
        for l in range(L):
            kprev = ki if l == 0 else kh
            tiles = []
            for k in range(kprev):
                w = wpool.tile([P, H], dt.float16, tag=f"w{l}{k}",
                               name=f"w{l}{k}")
                tiles.append(w)
            wt.append(tiles)
        for k in range(ki):
            nc.gpsimd.dma_start(out=wt[0][k][:], in_=w0t_d[k * P:(k + 1) * P, :])
        # all 16 per-(l,m) recurrent-weight columns in ONE [128,16] tile
        wbt = wpool.tile([P, L * m4], dt.float32, tag="wbt")
        nc.gpsimd.dma_start(out=wbt[:], in_=wbc_d)
        wbc = [[wbt[:, (l * m4 + m):(l * m4 + m) + 1] for m in range(m4)]
               for l in range(L)]
        for l in range(1, L):
            for k in range(kh):
                nc.gpsimd.dma_start(out=wt[l][k][:],
                                    in_=wst_d[l - 1, k * P:(k + 1) * P, :])
        if include_bias:
            bias = []
            for l in range(L):
                bt = wpool.tile([1, H], dt.float16, tag=f"b{l}")
                nc.gpsimd.dma_start(out=bt[:], in_=bias_d[l, :, :])
                bias.append(bt)
            ones = wpool.tile([1, TCH], dt.float16, tag="ones")
            nc.gpsimd.memset(ones[:], 1.0)
        # Non-PE instructions can carry only ONE sync-wait through walrus
        # codegen (probed: DVE scan/copy and ACT activation all fail with 2).
        # Same-engine waits merge into one semaphore, so the scheme is:
        # each engine touches every cross-engine dependency in a cheap
        # "absorber/claimer" op first, leaving the real op a single wait.
        # Preamble: DVE and ACT each touch every DMA-loaded scan operand so
        # later ops never need a DMA-queue wait.
        scratch = wpool.tile([P, L * m4], dt.float32, tag="scratch")
        scr_act = wpool.tile([P, L * m4], dt.float32, tag="scr_act")
        nc.vector.tensor_copy(scratch[:, 0:1], wbc[0][0])
        nc.scalar.activation(scr_act[:, 0:1], wbc[0][0], Act.Relu)
        # rotating per-tile scratch columns for the ACT claimer chain (a
        # fixed column would WAW against itself and add an ACT-own wait on
        # top of the DVE data wait)
        scr_rot = wpool.tile([P, 2 * bloc * L * m4], dt.float32, tag="scr_rot")
        scr_gp = wpool.tile([P, bloc * m4], dt.float32, tag="scr_gp")
        # PE preamble: junk ldweights per weight tile (no PSUM write, so no
        # WAW) so later real matmuls never carry a weight-DMA wait (PE is
        # also a 1-sync-wait engine).
        for l in range(L):
            for k in range(len(wt[l])):
                nc.tensor.ldweights(weights=wt[l][k][:, 0:P])
        if include_bias:
            for l in range(L):
                nc.tensor.ldweights(weights=bias[l][:, 0:P])
            nc.tensor.ldweights(weights=ones[:, 0:P])

        # --- main loop (layer-major: l outer, then batch, then m-tile) ---
        houts = {}
        xp_count = 0
        xp_readers = {}  # psum slot -> last scan1 instruction that read it
        o_readers = {}   # m -> out DMA of batch pair 0
        ti = 0
        for pair in range(bloc // 2):
          pair_b = (2 * pair, 2 * pair + 1)
          xtiles_b = {}
          for b in pair_b:
              xtiles = []
              for k in range(ki):
                  xt = xpool.tile([P, t], dt.float16, tag="xin")
                  nc.gpsimd.dma_start(out=xt[:],
                                      in_=xT_d[b, k * P:(k + 1) * P, :])
                  xtiles.append(xt)
              xtiles_b[b] = xtiles
          prev_b = xtiles_b
          for l in range(L):
            htiles_b = {b: [] for b in pair_b}
            for b in pair_b:
                prev = prev_b[b]
                for m in range(m4):
                    xp = psum.tile([P, t], dt.float32, tag="xp")
                    kprev = len(prev)
                    old_rd = xp_readers.get(xp_count % 2)
                    xp_count += 1
                    claimers = []
                    if old_rd is not None:
                        ldw = nc.tensor.ldweights(weights=wt[l][0][:, 0:P])
                        bass._add_dep_helper(
                            ldw.ins, old_rd.ins, sync=True,
                            reason="PE DVE-clock claimer for PSUM slot WAR")
                        claimers.append(ldw)
                    if m == 0:
                        for kc in range(kprev if l == 0 else 1):
                            claimers.append(nc.tensor.ldweights(
                                weights=prev[kprev - 1 - kc][:, 0:P]))
                    last_mm = None
                    for n in range(nch):
                        ns = slice(n * TCH, (n + 1) * TCH)
                        for k in range(kprev):
                            last_mm = nc.tensor.matmul(
                                xp[:, ns], lhsT=wt[l][k][:, m * P:(m + 1) * P],
                                rhs=prev[k][:, ns],
                                start=(k == 0),
                                stop=(k == kprev - 1 and not include_bias))
                            for cl in claimers:
                                bass._add_dep_helper(
                                    last_mm.ins, cl.ins, sync=False,
                                    reason="order claimer before real MMs")
                            claimers = []
                        if include_bias:
                            last_mm = nc.tensor.matmul(
                                xp[:, ns], lhsT=bias[l][:, m * P:(m + 1) * P],
                                rhs=ones[:, :], start=False, stop=True)
                    dlocb = dpool.tile([P, t + 2], dt.float16, tag="dloc")
                    nc.vector.memset(dlocb[:, 0:2], 0.0)
                    q = qpool.tile([P, t], dt.float16, tag="q")
                    wb_full = wbc[l][m].broadcast_to((P, t))
                    scan1 = nc.vector.tensor_tensor_scan(
                        out=dlocb[:, 2:t + 2],
                        data0=wb_full, data1=xp[:, 0:t],
                        initial=0.0, op0=Alu.mult, op1=Alu.add)
                    xp_readers[(xp_count - 1) % 2] = scan1
                    nc.vector.tensor_tensor_scan(
                        out=q[:, 0:t],
                        data0=dlocb[:, 1:t + 1], data1=wb_full,
                        initial=0.0, op0=Alu.min, op1=Alu.mult)
                    s = spool.tile([P, t], dt.float16, tag="s")
                    nc.vector.memset(s[:, 0:1], 0.0)
                    nc.vector.tensor_tensor(
                        out=s[:], in0=dlocb[:, 2:t + 2], in1=q[:],
                        op=Alu.subtract)
                    ti2 = 2 * ti
                    if l < L - 1:
                        h = hpool.tile([P, t], dt.float16, tag="h")
                        c0 = nc.scalar.activation(scr_rot[:, ti2:ti2 + 1],
                                                  wbc[l][m], Act.Relu)
                        bass._add_dep_helper(
                            c0.ins, last_mm.ins, sync=True,
                            reason="ACT PE-clock claimer for h slot WAR")
                        nc.scalar.activation(scr_rot[:, ti2 + 1:ti2 + 2],
                                             s[:, 0:1], Act.Relu)
                        nc.scalar.activation(h[:], s[:], Act.Relu)
                        htiles_b[b].append(h)
                    else:
                        if b % 2 == 0:
                            h2 = opool.tile([P, 2 * t], dt.float16,
                                            tag="hout")
                            houts[m] = h2
                        h2 = houts[m]
                        if b == 2:
                            c2 = nc.scalar.activation(
                                scr_rot[:, ti2:ti2 + 1],
                                wbc[l][m], Act.Relu)
                            bass._add_dep_helper(
                                c2.ins, o_readers[m].ins, sync=True,
                                reason="ACT DMA-clock claimer for staging WAR")
                        c1 = nc.scalar.activation(
                            scr_rot[:, ti2 + 1:ti2 + 2], s[:, 0:1], Act.Relu)
                        relu = nc.scalar.activation(
                            h2[:, (b % 2) * t:(b % 2 + 1) * t], s[:],
                            Act.Relu)
                        pins_a = [c1] + ([c2] if b == 2 else [])
                        for cc in pins_a:
                            bass._add_dep_helper(
                                relu.ins, cc.ins, sync=False,
                                reason="claimers before relu")
                        if b % 2 == 1:
                            dst = out_d[b - 1:b + 1, m * P:(m + 1) * P, :]
                            dma = nc.sync.dma_start(
                                out=dst.rearrange("b p t -> p b t"),
                                in_=h2[:].rearrange("p (b t) -> p b t", b=2))
                            o_readers[m] = dma
                    ti += 1
            prev_b = htiles_b
        scan1_last = scan1
        # Tail pre-drain: the auto kernel-tail drain on SP must observe
        # every DMA queue and engine tick; feed SP one dependency per
        # pre-drain (same-proc waits merge) so the auto drain ends at zero.
        tail_deps = [i for i in nc.inst_map.values()
                     if type(i).__name__ == "InstDMACopy"]
        tail_deps += [last_mm.ins, scan1_last.ins, relu.ins]
        for depi in tail_deps:
            dr = nc.sync.drain(fusable=False)
            bass._add_dep_helper(dr.ins, depi, sync=True,
                                 reason="tail pre-drain absorber")
    _assert_wait_budget(nc)
    return nc


# Instruction families exempt from the 1-sync-wait TPB events header (DMA
# descriptors and drains use the queue sync machinery). Everything that runs
# on a TPB engine sequencer (PE/DVE/ACT/Pool alike) is capacity-1.
_MULTI_WAIT_OK = {"InstDrain",
                  "InstEventSemaphore", "InstUnconditionalBranch",
                  "InstRegisterMove", "InstISA", "InstTensorLoad",
                  "InstTensorSave"}


def _assert_wait_budget(nc):
    bad = []
    for name, inst in nc.inst_map.items():
        ty = type(inst).__name__
        if ty in _MULTI_WAIT_OK:
            continue
        w = inst.sync_info.on_wait if inst.sync_info else []
        if len(w) > 1:
            bad.append((name, ty,
                        [f"{x.ant_name}>={x.wait_value}" for x in w]))
    if bad:
        raise RuntimeError(
            f"{len(bad)} instructions exceed the 1-sync-wait TPB limit, "
            f"first few: {bad[:5]}")


def _prep_core_inputs(Input, W0, Ws, bs, whs, core):
    """Host-side staging for one core: shard batch, transpose layer-0 input,
    pre-transpose weights into lhsT layout, broadcast recurrent weights."""
    bsl = slice(core * BLOC, (core + 1) * BLOC)
    return {
        "xT": np.ascontiguousarray(
            Input[bsl].transpose(0, 2, 1)).astype(np.float16),
        "w0t": np.ascontiguousarray(W0.T).astype(np.float16),
        "wst": np.ascontiguousarray(Ws.transpose(0, 2, 1)).astype(np.float16),
        "bias": np.ascontiguousarray(bs[:, None, :]).astype(np.float16),
        "wbc": np.ascontiguousarray(
            whs.astype(np.float32).reshape(L, H // P, P).transpose(2, 0, 1)
            .reshape(P, L * (H // P))),
    }


def kernel(Input, W0, Ws, bs, whs):
    include_bias = bool(np.any(bs != 0))
    nc = build(include_bias=include_bias)
    in_maps = [_prep_core_inputs(Input, W0, Ws, bs, whs, r)
               for r in range(NCORES)]
    res = run_bass_kernel_spmd(nc, in_maps, core_ids=list(range(NCORES)))
    parts = [res.results[r]["out"] for r in range(NCORES)]  # [BLOC, H, T] each
    full = np.concatenate(parts, axis=0)  # [B, H, T]
    return np.ascontiguousarray(full.transpose(0, 2, 1)).astype(np.float32)



# revision 3
# speedup vs baseline: 1.0130x; 1.0130x over previous
"""Trainium2 Bass kernel v3 for a 4-layer IndRNN (B=32, T=2048, I=256, H=512).

v5 = v4 + startup trim: wbc packed into one [128, L*m4] tile (1 DMA instead
of 16), and input DMAs for the first batch pair issued before the later-layer
weight loads so the first scan starts ~20us earlier.
v4 = v3 + PAIR-wise layer-major tile order (for batch-pair: for l: for b
in pair: for m). The per-layer serialization chain relu(l, m=3) -> matmul
group -> scan1(l+1, m=0) is hidden behind the partner batch's ~40us of DVE
work, removing ~100us of DVE idle at layer boundaries, while h tiles span
only 2 batches x 2 layer generations (16 bufs). v3: TT-subtract combine
(DVE 2x) + ACT final relu.

Math: per layer, xp = x @ W.T + b, then the per-channel recurrence
    h_t = relu(xp_t + w * h_{t-1}),  w = whs[l] in [0, 1)

Since w >= 0, the nonlinear scan decomposes into two linear-style scans that
map 1:1 onto the DVE `tensor_tensor_scan` instruction:
    dloc_t = w * dloc_{t-1} + xp_t          (unclamped linear scan)
    q_t    = w * min(dloc_{t-1}, q_{t-1})   (min-scan; dloc_{-1} = q_{-1} = 0)
    h_t    = relu(dloc_t - q_t)
Proof sketch: with s_t = xp_t + w*relu(s_{t-1}) (so h_t = relu(s_t)),
s_t = max(xp_t, xp_t + w*s_{t-1}) for w >= 0; substituting s_t = dloc_t + r_t
gives r_t = max(-w*dloc_{t-1}, w*r_{t-1}), i.e. q_t = -r_t satisfies the
min-scan above. Verified exactly in fp64 against the sequential reference.

Sharding: data-parallel over batch, 4 batches per core, weights replicated.
Layout on device: [H(partitions), T(free)] per batch; the host pre-transposes
the layer-0 input to [I, T] and post-transposes the output from [H, T], so the
device never pays for transposes.
"""

import numpy as np
from contextlib import ExitStack

import concourse.bass as bass
import concourse.tile as tile
from concourse import mybir
from concourse.bass_utils import run_bass_kernel_spmd

dt = mybir.dt
Alu = mybir.AluOpType
Act = mybir.ActivationFunctionType

B, T, I, H, L = 32, 2048, 256, 512, 4
NCORES = 8
BLOC = B // NCORES
P = 128
TCH = 512  # time chunk = one PSUM bank of fp32


def build(bloc=BLOC, t=T, include_bias=False, trace_sim=False,
          whole_t_scan=True, bcast_w=True, gpsimd_stt=False, act_relu=True):
    """Build the per-core Bass program (SPMD; identical on all cores)."""
    assert t % TCH == 0
    nch = t // TCH
    ki, kh, m4 = I // P, H // P, H // P

    nc = bass.Bass("TRN2", target_bir_lowering=False, debug=False,
                   num_devices=NCORES)
    xT_d = nc.dram_tensor("xT", [bloc, I, t], dt.float16, kind="ExternalInput").ap()
    w0t_d = nc.dram_tensor("w0t", [I, H], dt.float16, kind="ExternalInput").ap()
    wst_d = nc.dram_tensor("wst", [L - 1, H, H], dt.float16, kind="ExternalInput").ap()
    bias_d = nc.dram_tensor("bias", [L, 1, H], dt.float16, kind="ExternalInput").ap()
    wbc_d = nc.dram_tensor("wbc", [P, L * (H // P)], dt.float32, kind="ExternalInput").ap()
    out_d = nc.dram_tensor("out", [bloc, H, t], dt.float16, kind="ExternalOutput").ap()

    with tile.TileContext(nc, trace_sim=trace_sim) as tc, ExitStack() as ctx:
        wpool = ctx.enter_context(tc.tile_pool(name="weights", bufs=1))
        xpool = ctx.enter_context(tc.tile_pool(name="xin", bufs=2 * BLOC))
        hpool = ctx.enter_context(tc.tile_pool(name="h", bufs=16))
        dpool = ctx.enter_context(tc.tile_pool(name="dloc", bufs=3))
        qpool = ctx.enter_context(tc.tile_pool(name="q", bufs=3))
        spool = ctx.enter_context(tc.tile_pool(name="s", bufs=3))
        opool = ctx.enter_context(tc.tile_pool(name="hout", bufs=4))
        psum = ctx.enter_context(tc.tile_pool(name="psum", bufs=2, space="PSUM"))

        # --- persistent weights ---
        # layer-0 weights first so the first matmul group can start ASAP;
        # later-layer weights stream in behind it.
        wt = []  # wt[l][k] -> [128, H] fp16
        for l in range(L):
            kprev = ki if l == 0 else kh
            tiles = []
            for k in range(kprev):
                w = wpool.tile([P, H], dt.float16, tag=f"w{l}{k}",
                               name=f"w{l}{k}")
                tiles.append(w)
            wt.append(tiles)
        for k in range(ki):
            nc.gpsimd.dma_start(out=wt[0][k][:], in_=w0t_d[k * P:(k + 1) * P, :])
        # all 16 per-(l,m) recurrent-weight columns in ONE [128,16] tile
        wbt = wpool.tile([P, L * m4], dt.float32, tag="wbt")
        nc.gpsimd.dma_start(out=wbt[:], in_=wbc_d)
        wbc = [[wbt[:, (l * m4 + m):(l * m4 + m) + 1] for m in range(m4)]
               for l in range(L)]
        # first batch-pair inputs BEFORE the later-layer weights: wst (3MB)
        # is not needed until ~100us in, xT pair0 gates the first scan.
        all_xtiles = {}
        for b in range(bloc):
            all_xtiles[b] = [xpool.tile([P, t], dt.float16, tag="xin",
                                        name=f"x{b}{k}") for k in range(ki)]
        for b in (0, 1):
            for k in range(ki):
                nc.gpsimd.dma_start(out=all_xtiles[b][k][:],
                                    in_=xT_d[b, k * P:(k + 1) * P, :])
        for l in range(1, L):
            for k in range(kh):
                nc.gpsimd.dma_start(out=wt[l][k][:],
                                    in_=wst_d[l - 1, k * P:(k + 1) * P, :])
        for b in (2, 3):
            for k in range(ki):
                nc.gpsimd.dma_start(out=all_xtiles[b][k][:],
                                    in_=xT_d[b, k * P:(k + 1) * P, :])
        if include_bias:
            bias = []
            for l in range(L):
                bt = wpool.tile([1, H], dt.float16, tag=f"b{l}")
                nc.gpsimd.dma_start(out=bt[:], in_=bias_d[l, :, :])
                bias.append(bt)
            ones = wpool.tile([1, TCH], dt.float16, tag="ones")
            nc.gpsimd.memset(ones[:], 1.0)
        # Non-PE instructions can carry only ONE sync-wait through walrus
        # codegen (probed: DVE scan/copy and ACT activation all fail with 2).
        # Same-engine waits merge into one semaphore, so the scheme is:
        # each engine touches every cross-engine dependency in a cheap
        # "absorber/claimer" op first, leaving the real op a single wait.
        # Preamble: DVE and ACT each touch every DMA-loaded scan operand so
        # later ops never need a DMA-queue wait.
        scratch = wpool.tile([P, L * m4], dt.float32, tag="scratch")
        scr_act = wpool.tile([P, L * m4], dt.float32, tag="scr_act")
        nc.vector.tensor_copy(scratch[:, 0:1], wbc[0][0])
        nc.scalar.activation(scr_act[:, 0:1], wbc[0][0], Act.Relu)
        # rotating per-tile scratch columns for the ACT claimer chain (a
        # fixed column would WAW against itself and add an ACT-own wait on
        # top of the DVE data wait)
        scr_rot = wpool.tile([P, 2 * bloc * L * m4], dt.float32, tag="scr_rot")
        scr_gp = wpool.tile([P, bloc * m4], dt.float32, tag="scr_gp")
        # PE preamble: junk ldweights per weight tile (no PSUM write, so no
        # WAW) so later real matmuls never carry a weight-DMA wait (PE is
        # also a 1-sync-wait engine).
        for l in range(L):
            for k in range(len(wt[l])):
                nc.tensor.ldweights(weights=wt[l][k][:, 0:P])
        if include_bias:
            for l in range(L):
                nc.tensor.ldweights(weights=bias[l][:, 0:P])
            nc.tensor.ldweights(weights=ones[:, 0:P])

        # --- main loop (layer-major: l outer, then batch, then m-tile) ---
        houts = {}
        xp_count = 0
        xp_readers = {}  # psum slot -> last scan1 instruction that read it
        o_readers = {}   # m -> out DMA of batch pair 0
        ti = 0
        for pair in range(bloc // 2):
          pair_b = (2 * pair, 2 * pair + 1)
          prev_b = {b: all_xtiles[b] for b in pair_b}
          for l in range(L):
            htiles_b = {b: [] for b in pair_b}
            for b in pair_b:
                prev = prev_b[b]
                for m in range(m4):
                    xp = psum.tile([P, t], dt.float32, tag="xp")
                    kprev = len(prev)
                    old_rd = xp_readers.get(xp_count % 2)
                    xp_count += 1
                    claimers = []
                    if old_rd is not None:
                        ldw = nc.tensor.ldweights(weights=wt[l][0][:, 0:P])
                        bass._add_dep_helper(
                            ldw.ins, old_rd.ins, sync=True,
                            reason="PE DVE-clock claimer for PSUM slot WAR")
                        claimers.append(ldw)
                    if m == 0:
                        for kc in range(kprev if l == 0 else 1):
                            claimers.append(nc.tensor.ldweights(
                                weights=prev[kprev - 1 - kc][:, 0:P]))
                    last_mm = None
                    for n in range(nch):
                        ns = slice(n * TCH, (n + 1) * TCH)
                        for k in range(kprev):
                            last_mm = nc.tensor.matmul(
                                xp[:, ns], lhsT=wt[l][k][:, m * P:(m + 1) * P],
                                rhs=prev[k][:, ns],
                                start=(k == 0),
                                stop=(k == kprev - 1 and not include_bias))
                            for cl in claimers:
                                bass._add_dep_helper(
                                    last_mm.ins, cl.ins, sync=False,
                                    reason="order claimer before real MMs")
                            claimers = []
                        if include_bias:
                            last_mm = nc.tensor.matmul(
                                xp[:, ns], lhsT=bias[l][:, m * P:(m + 1) * P],
                                rhs=ones[:, :], start=False, stop=True)
                    dlocb = dpool.tile([P, t + 2], dt.float16, tag="dloc")
                    nc.vector.memset(dlocb[:, 0:2], 0.0)
                    q = qpool.tile([P, t], dt.float16, tag="q")
                    wb_full = wbc[l][m].broadcast_to((P, t))
                    scan1 = nc.vector.tensor_tensor_scan(
                        out=dlocb[:, 2:t + 2],
                        data0=wb_full, data1=xp[:, 0:t],
                        initial=0.0, op0=Alu.mult, op1=Alu.add)
                    xp_readers[(xp_count - 1) % 2] = scan1
                    nc.vector.tensor_tensor_scan(
                        out=q[:, 0:t],
                        data0=dlocb[:, 1:t + 1], data1=wb_full,
                        initial=0.0, op0=Alu.min, op1=Alu.mult)
                    s = spool.tile([P, t], dt.float16, tag="s")
                    nc.vector.memset(s[:, 0:1], 0.0)
                    nc.vector.tensor_tensor(
                        out=s[:], in0=dlocb[:, 2:t + 2], in1=q[:],
                        op=Alu.subtract)
                    ti2 = 2 * ti
                    if l < L - 1:
                        h = hpool.tile([P, t], dt.float16, tag="h")
                        c0 = nc.scalar.activation(scr_rot[:, ti2:ti2 + 1],
                                                  wbc[l][m], Act.Relu)
                        bass._add_dep_helper(
                            c0.ins, last_mm.ins, sync=True,
                            reason="ACT PE-clock claimer for h slot WAR")
                        nc.scalar.activation(scr_rot[:, ti2 + 1:ti2 + 2],
                                             s[:, 0:1], Act.Relu)
                        nc.scalar.activation(h[:], s[:], Act.Relu)
                        htiles_b[b].append(h)
                    else:
                        if b % 2 == 0:
                            h2 = opool.tile([P, 2 * t], dt.float16,
                                            tag="hout")
                            houts[m] = h2
                        h2 = houts[m]
                        if b == 2:
                            c2 = nc.scalar.activation(
                                scr_rot[:, ti2:ti2 + 1],
                                wbc[l][m], Act.Relu)
                            bass._add_dep_helper(
                                c2.ins, o_readers[m].ins, sync=True,
                                reason="ACT DMA-clock claimer for staging WAR")
                        c1 = nc.scalar.activation(
                            scr_rot[:, ti2 + 1:ti2 + 2], s[:, 0:1], Act.Relu)
                        relu = nc.scalar.activation(
                            h2[:, (b % 2) * t:(b % 2 + 1) * t], s[:],
                            Act.Relu)
                        pins_a = [c1] + ([c2] if b == 2 else [])
                        for cc in pins_a:
                            bass._add_dep_helper(
                                relu.ins, cc.ins, sync=False,
                                reason="claimers before relu")
                        if b % 2 == 1:
                            dst = out_d[b - 1:b + 1, m * P:(m + 1) * P, :]
                            dma = nc.sync.dma_start(
                                out=dst.rearrange("b p t -> p b t"),
                                in_=h2[:].rearrange("p (b t) -> p b t", b=2))
                            o_readers[m] = dma
                    ti += 1
            prev_b = htiles_b
        scan1_last = scan1
        # Tail pre-drain: the auto kernel-tail drain on SP must observe
        # every DMA queue and engine tick; feed SP one dependency per
        # pre-drain (same-proc waits merge) so the auto drain ends at zero.
        tail_deps = [i for i in nc.inst_map.values()
                     if type(i).__name__ == "InstDMACopy"]
        tail_deps += [last_mm.ins, scan1_last.ins, relu.ins]
        for depi in tail_deps:
            dr = nc.sync.drain(fusable=False)
            bass._add_dep_helper(dr.ins, depi, sync=True,
                                 reason="tail pre-drain absorber")
    _assert_wait_budget(nc)
    return nc


# Instruction families exempt from the 1-sync-wait TPB events header (DMA
# descriptors and drains use the queue sync machinery). Everything that runs
# on a TPB engine sequencer (PE/DVE/ACT/Pool alike) is capacity-1.
_MULTI_WAIT_OK = {"InstDrain",
                  "InstEventSemaphore", "InstUnconditionalBranch",
                  "InstRegisterMove", "InstISA", "InstTensorLoad",
                  "InstTensorSave"}


def _assert_wait_budget(nc):
    bad = []
    for name, inst in nc.inst_map.items():
        ty = type(inst).__name__
        if ty in _MULTI_WAIT_OK:
            continue
        w = inst.sync_info.on_wait if inst.sync_info else []
        if len(w) > 1:
            bad.append((name, ty,
                        [f"{x.ant_name}>={x.wait_value}" for x in w]))
    if bad:
        raise RuntimeError(
            f"{len(bad)} instructions exceed the 1-sync-wait TPB limit, "
            f"first few: {bad[:5]}")


def _prep_core_inputs(Input, W0, Ws, bs, whs, core):
    """Host-side staging for one core: shard batch, transpose layer-0 input,
    pre-transpose weights into lhsT layout, broadcast recurrent weights."""
    bsl = slice(core * BLOC, (core + 1) * BLOC)
    return {
        "xT": np.ascontiguousarray(
            Input[bsl].transpose(0, 2, 1)).astype(np.float16),
        "w0t": np.ascontiguousarray(W0.T).astype(np.float16),
        "wst": np.ascontiguousarray(Ws.transpose(0, 2, 1)).astype(np.float16),
        "bias": np.ascontiguousarray(bs[:, None, :]).astype(np.float16),
        "wbc": np.ascontiguousarray(
            whs.astype(np.float32).reshape(L, H // P, P).transpose(2, 0, 1)
            .reshape(P, L * (H // P))),
    }


def kernel(Input, W0, Ws, bs, whs):
    include_bias = bool(np.any(bs != 0))
    nc = build(include_bias=include_bias)
    in_maps = [_prep_core_inputs(Input, W0, Ws, bs, whs, r)
               for r in range(NCORES)]
    res = run_bass_kernel_spmd(nc, in_maps, core_ids=list(range(NCORES)))
    parts = [res.results[r]["out"] for r in range(NCORES)]  # [BLOC, H, T] each
    full = np.concatenate(parts, axis=0)  # [B, H, T]
    return np.ascontiguousarray(full.transpose(0, 2, 1)).astype(np.float32)

